# revision 21
# baseline (speedup 1.0000x reference)
"""Pooled-KV attention block on 8 Trainium2 cores, data-parallel over batch.

Reference computation (per batch element b, with x_b: [64, 64, 512] -> [4096, 512]):
    f  = x_b @ wf                     # [4096, 64]
    xp = avgpool2x2(x_b)              # [1024, 512]
    g  = xp @ wg                      # [1024, 64]
    h  = xp @ wh                      # [1024, 256]
    a  = softmax(f @ g.T, axis=-1)    # [4096, 1024]
    y  = a @ h                        # [4096, 256]
    out = y @ wo                      # [4096, 512]

Kernel strategy (one core per batch element, weights replicated):
  - Host supplies x transposed and cast to fp16 (xT: [512, 4096]) so channel
    contractions have C on SBUF partitions; no on-device transposes anywhere.
  - All intermediates flow "transposed": fT [64, 4096], gT [64, 1024],
    h [m, 256] with m on partitions, scoresT [m, n], yT [e, n].
  - Matmul operands are fp16 (full PE rate, 10-bit mantissa, fp32 PSUM
    accumulation); final output is fp16 (upcast to fp32 on host).
  - Softmax skips max-subtraction (|scores| < ~6 for this data, exp is safe).
  - Row sums: exp tiles are pair-accumulated on DVE into acc [m,512], then
    four tiny N=1 matmuls (lhsT=acc slice, rhs=ones column) produce the sums
    already transposed to [n_chunk, 1] per-partition form -- no wide ones
    matmul and no DRAM transpose bounce.  Normalization stays folded into
    the out-projection copyback as a per-partition scalar multiply.
"""

import sys
import types

import numpy as np

import concourse.mybir as mybir
import concourse.tile as tile
from concourse import bacc
from concourse.bass_utils import run_bass_kernel_spmd

# If BASS_TRACE is set but this image's antenv lacks axon_hooks, bass_utils
# would crash on import; provide a no-op hook module so tracing degrades
# gracefully instead (a real hook installed earlier, e.g. by test.py, wins).
try:
    import antenv.axon_hooks  # noqa: F401
except ImportError:
    import antenv

    _stub = types.ModuleType("antenv.axon_hooks")
    _stub._hook = None
    _stub.set_axon_ntff_profile_hook = lambda h: setattr(_stub, "_hook", h)
    _stub.get_axon_ntff_profile_hook = lambda: _stub._hook
    sys.modules["antenv.axon_hooks"] = _stub
    antenv.axon_hooks = _stub

F32 = mybir.dt.float32
F16 = mybir.dt.float16

P = 128          # SBUF partitions
C = 512          # channels
KC = C // P      # 4 contraction chunks over channels
N = 4096         # query positions (64*64)
NTILE = 512      # n tile (psum free dim)
NT = N // NTILE  # 8 n tiles
M = 1024         # pooled key positions (32*32)
MC = M // P      # 8 key chunks
D = 64           # qk head dim
E = 256          # value dim (C//2)
EC = E // P      # 2 value chunks

_CACHE = {}


def _build():
    nc = bacc.Bacc(None, target_bir_lowering=False)

    xt_d = nc.dram_tensor("xt", [4, C, N // 4], F16, kind="ExternalInput")
    wf_d = nc.dram_tensor("wf2", [C, P], F16, kind="ExternalInput")   # [wf | wf]
    wg_d = nc.dram_tensor("wg2", [C, P], F16, kind="ExternalInput")   # 0.25*[wg | wg]
    wh_d = nc.dram_tensor("whs", [C, E], F16, kind="ExternalInput")   # 0.25*wh
    wo_d = nc.dram_tensor("wo", [E, C], F16, kind="ExternalInput")
    out_d = nc.dram_tensor("out", [N, C], F16, kind="ExternalOutput")

    with tile.TileContext(nc) as tc:
        with (
            tc.tile_pool(name="const", bufs=1) as const_pool,
            tc.tile_pool(name="ptmp", bufs=4) as ptmp_pool,
            tc.tile_pool(name="exp", bufs=4) as exp_pool,
            tc.tile_pool(name="sacc", bufs=2) as sacc_pool,
            tc.tile_pool(name="ysb", bufs=2) as y_pool,
            tc.tile_pool(name="osb", bufs=6) as o_pool,
            tc.tile_pool(name="small", bufs=4) as small_pool,
            tc.tile_pool(name="ps_pair", bufs=2, space="PSUM") as ps_pair_pool,
            tc.tile_pool(name="ps_work", bufs=1, space="PSUM") as ps_work_pool,
            tc.tile_pool(name="ps_y", bufs=1, space="PSUM") as ps_y_pool,
            tc.tile_pool(name="ps_rsum", bufs=1, space="PSUM") as ps_rsum_pool,
            tc.tile_pool(name="warm", bufs=1) as warm_pool,
        ):
            # ---- staged input load + f/g/h, by n-quarters of x ----
            # Each quarter of x (all channels, 1024 query positions) enables:
            # its pooling slice, two fT tiles, one gT quarter, two h chunks.
            # PE work starts when the first quarter lands instead of after the
            # full x load; DMA issue round-robins four HWDGE rings.
            xt_q = []
            for q in range(4):
                t = const_pool.tile([P, KC, N // 4], F16, name=f"xt_q{q}")
                xt_q.append(t)
            xp_q = []
            for q in range(4):
                t = const_pool.tile([P, KC, M // 4], F16, name=f"xp_q{q}")
                xp_q.append(t)
            wf_sb = const_pool.tile([P, KC, P], F16)
            wg_sb = const_pool.tile([P, KC, P], F16)
            wh_sb = const_pool.tile([P, KC, E], F16)
            wo_sb = const_pool.tile([P, EC, C], F16)
            ones_sb = const_pool.tile([P, 1], F16)
            fT_sb = const_pool.tile([P, N], F16)
            gT_sb = const_pool.tile([P, M], F16)
            h_sb = const_pool.tile([P, MC, E], F16)

            nc.gpsimd.memset(ones_sb, 1.0)
            warm_sb = warm_pool.tile([P, NTILE], F16)
            nc.gpsimd.memset(warm_sb, 1.0)

            NP = MC // 2  # score pairs per n tile

            ocur = {}

            def out_chunk(y_prev, recip_prev, nt_prev, j):
                # out chunks are produced in j pairs sharing one o_sb tile so
                # each store is a single wide DMA descriptor (two row blocks)
                ps_o = ps_work_pool.tile([P, C], F32, tag="ps_work", name=f"ps_o_{nt_prev}_{j}")
                for ec in range(EC):
                    nc.tensor.matmul(
                        ps_o,
                        lhsT=y_prev[:, ec, j * P : (j + 1) * P],
                        rhs=wo_sb[:, ec, :],
                        start=(ec == 0),
                        stop=(ec == EC - 1),
                    )
                if j % 2 == 0:
                    ocur["t"] = o_pool.tile([P, 2, C], F16, tag="o_sb",
                                            name=f"o2_{nt_prev}_{j}")
                o_sb = ocur["t"]
                nc.vector.tensor_scalar_mul(
                    o_sb[:, j % 2, :], ps_o, recip_prev[:, j : j + 1]
                )
                if j % 2 == 1:
                    row0 = nt_prev * NTILE + (j - 1) * P
                    nc.sync.dma_start(
                        out_d[row0 : row0 + 2 * P, :].rearrange(
                            "(k p) c -> p k c", p=P
                        ),
                        o_sb,
                    )

            class TileState:
                pass

            def attn_begin(nt):
                st = TileState()
                st.nt = nt
                st.ps_y = ps_y_pool.tile([P, 2 * NTILE], F32, tag="ps_y", name=f"ps_y_{nt}")
                st.ets = {}
                st.et2s = {}
                return st

            def attn_scores(st, mc2):
                # two K=64 score matmuls packed into disjoint row groups,
                # writing the two banks of one psum tile; one wide exp
                nt = st.nt
                nsl = slice(nt * NTILE, (nt + 1) * NTILE)
                mcA, mcB = 2 * mc2, 2 * mc2 + 1
                ps_s2 = ps_pair_pool.tile([P, 2 * NTILE], F32, tag="ps_pair", name=f"ps_s2_{nt}_{mc2}")
                # 64x128 row tiling: the two K=64 matmuls run on independent
                # PE row-tiles T0 (SBUF 0-63) / T8 (SBUF 64-127) concurrently
                nc.tensor.matmul(
                    ps_s2[:, :NTILE],
                    lhsT=gT_sb[0:D, mcA * P : (mcA + 1) * P],
                    rhs=fT_sb[0:D, nsl],
                    start=True, stop=True,
                    tile_position=(0, 0),
                )
                nc.tensor.matmul(
                    ps_s2[:, NTILE:],
                    lhsT=gT_sb[D : 2 * D, mcB * P : (mcB + 1) * P],
                    rhs=fT_sb[D : 2 * D, nsl],
                    start=True, stop=True,
                    tile_position=(D, 0),
                )
                et2 = exp_pool.tile([P, 2 * NTILE], F16, tag="et", name=f"et2_{nt}_{mc2}")
                nc.scalar.activation(et2, ps_s2, mybir.ActivationFunctionType.Exp)
                st.ets[mc2] = (et2[:, :NTILE], et2[:, NTILE:])
                st.et2s[mc2] = et2

            def attn_consume(st, pc):
                first = pc == 0
                last = pc == NP - 1
                for k, et in enumerate(st.ets.pop(pc)):
                    mc = 2 * pc + k
                    nc.tensor.matmul(
                        st.ps_y[:, :NTILE], lhsT=h_sb[:, mc, 0:P], rhs=et,
                        start=first and k == 0, stop=last and k == 1,
                    )
                    nc.tensor.matmul(
                        st.ps_y[:, NTILE:], lhsT=h_sb[:, mc, P:E], rhs=et,
                        start=first and k == 0, stop=last and k == 1,
                    )

            def sums_finish(st):
                # t23 = et2 + et3; s2 = t01 + t23; acc = s2_lo + s2_hi
                nt = st.nt
                t23 = sacc_pool.tile([P, 2 * NTILE], F16, tag="t23", name=f"t23_{nt}")
                nc.vector.tensor_add(t23, st.et2s.pop(2), st.et2s.pop(3))
                s2 = sacc_pool.tile([P, 2 * NTILE], F16, tag="s2", name=f"s2_{nt}")
                nc.vector.tensor_add(s2, st.t01, t23)
                acc = sacc_pool.tile([P, NTILE], F16, tag="acc", name=f"acc_{nt}")
                nc.vector.tensor_add(acc, s2[:, :NTILE], s2[:, NTILE:])
                st.acc = acc

            def rsum_mms(st):
                # transposed row sums: rsum[n_j, 0] = sum_m acc[m, n_j]
                nt = st.nt
                ps_rs = ps_rsum_pool.tile([P, NTILE // P], F32, tag="ps_rsum", name=f"ps_rs_{nt}")
                for j in range(NTILE // P):
                    nc.tensor.matmul(
                        ps_rs[:, j : j + 1],
                        lhsT=st.acc[:, j * P : (j + 1) * P],
                        rhs=ones_sb,
                        start=True, stop=True,
                    )
                recip = small_pool.tile([P, NTILE // P], F32, tag="recip")
                nc.vector.reciprocal(recip, ps_rs)
                return recip

            def attn_end(st):
                # one wide psum->sbuf copy spanning both y banks
                y_sb = y_pool.tile([P, EC, NTILE], F16, tag="y_sb")
                nc.scalar.copy(y_sb.rearrange("p e n -> p (e n)"), st.ps_y)
                return y_sb

            NQ = N // 4   # 1024 query positions per quarter
            MQ = M // 4   # 128 pooled positions per quarter

            def load_quarter(q):
                # each quarter split across both HWDGE rings, one wide
                # descriptor per ring (dma_start issue cost is ~0.6 us each);
                # host supplies x quarter-contiguous so reads are sequential
                nc.sync.dma_start(
                    xt_q[q][:, 0:2, :],
                    xt_d[q, 0 : 2 * P, :].rearrange("(kc p) n -> p kc n", p=P),
                )
                nc.scalar.dma_start(
                    xt_q[q][:, 2:4, :],
                    xt_d[q, 2 * P : 4 * P, :].rearrange("(kc p) n -> p kc n", p=P),
                )

            # small weights first (first f/g matmuls need them); wh/wo are not
            # needed until pooling / the first out-projection, so they ride
            # the slow gpsimd software-DGE ring in parallel with x
            nc.sync.dma_start(wf_sb, wf_d.rearrange("(kc p) d -> p kc d", p=P))
            nc.scalar.dma_start(wg_sb, wg_d.rearrange("(kc p) d -> p kc d", p=P))
            nc.gpsimd.dma_start(wh_sb, wh_d.rearrange("(kc p) e -> p kc e", p=P))
            nc.gpsimd.dma_start(wo_sb, wo_d.rearrange("(ec p) c -> p ec c", p=P))
            for q in range(4):
                load_quarter(q)

            # PE warmup: the first ~9 us are DMA-dead (ring init + x in
            # flight); dummy matmuls keep the HAM activity monitor from
            # holding the PE at half clock when real work arrives
            ps_warm = ps_work_pool.tile([P, C], F32, tag="ps_work", name="ps_warm")
            for w in range(34):
                nc.tensor.matmul(
                    ps_warm, lhsT=warm_sb[:, 0:P], rhs=warm_sb,
                    start=True, stop=True,
                )

            for q in range(4):
                # pooling for quarter q: n = 256*i + 64*a + 2*j + b over all kc
                # t0 on DVE, t1 on GpSimd so the two halves run concurrently
                xv = xt_q[q].rearrange(
                    "p kc (i a j b) -> p kc i a j b", i=8, a=2, j=32, b=2
                )
                t0 = ptmp_pool.tile([P, KC, 8, 32], F16, tag="pool_t0")
                nc.vector.tensor_add(t0, xv[:, :, :, 0, :, 0], xv[:, :, :, 0, :, 1])
                t1 = ptmp_pool.tile([P, KC, 8, 32], F16, tag="pool_t1")
                nc.vector.tensor_add(t1, xv[:, :, :, 1, :, 0], xv[:, :, :, 1, :, 1])
                nc.vector.tensor_add(
                    xp_q[q].rearrange("p kc (i j) -> p kc i j", i=8), t0, t1
                )

                # fT tiles for this quarter (two n tiles of 512)
                for half in range(2):
                    nt = 2 * q + half
                    ps_w = ps_pair_pool.tile([P, 2 * NTILE], F32, tag="ps_pair")
                    ps = ps_w[:, :NTILE]
                    for kc in range(KC):
                        nc.tensor.matmul(
                            ps,
                            lhsT=wf_sb[:, kc, :],
                            rhs=xt_q[q][:, kc, half * NTILE : (half + 1) * NTILE],
                            start=(kc == 0),
                            stop=(kc == KC - 1),
                        )
                    nc.scalar.copy(fT_sb[:, nt * NTILE : (nt + 1) * NTILE], ps)

                # gT quarter (128 key columns)
                ps_w = ps_pair_pool.tile([P, 2 * NTILE], F32, tag="ps_pair")
                ps = ps_w[:, :MQ]
                for kc in range(KC):
                    nc.tensor.matmul(
                        ps,
                        lhsT=wg_sb[:, kc, :],
                        rhs=xp_q[q][:, kc, :],
                        start=(kc == 0),
                        stop=(kc == KC - 1),
                    )
                nc.scalar.copy(gT_sb[:, q * MQ : (q + 1) * MQ], ps)

                # h chunks for this quarter (mc = 2q, 2q+1)
                for half in range(2):
                    mc = 2 * q + half
                    ps_w = ps_pair_pool.tile([P, 2 * NTILE], F32, tag="ps_pair")
                    ps = ps_w[:, :E]
                    for kc in range(KC):
                        nc.tensor.matmul(
                            ps,
                            lhsT=xp_q[q][:, kc, half * P : (half + 1) * P],
                            rhs=wh_sb[:, kc, :],
                            start=(kc == 0),
                            stop=(kc == KC - 1),
                        )
                    nc.scalar.copy(h_sb[:, mc, :], ps)

            # ---- attention, software-pipelined ----
            # Tile nt's scores/exp/y run while tile nt-1 finishes: its
            # transposed row-sum mini-matmuls + reciprocal land at the top of
            # tile nt, and its four out-projection chunks interleave between
            # tile nt's score/consume steps.

            prev = None
            for nt in range(NT):
                st = attn_begin(nt)
                for mc2 in range(NP + 1):
                    if mc2 < NP:
                        attn_scores(st, mc2)
                    if mc2 == 0 and prev is not None:
                        prev_recip = rsum_mms(prev[0])
                        prev = (prev[0], prev[1], prev_recip)
                    if mc2 >= 1:
                        attn_consume(st, mc2 - 1)
                        if mc2 == 2:
                            t01 = sacc_pool.tile([P, 2 * NTILE], F16, tag="t01", name=f"t01_{nt}")
                            nc.vector.tensor_add(t01, st.et2s.pop(0), st.et2s.pop(1))
                            st.t01 = t01
                        if prev is not None and 1 <= mc2 <= NP - 1:
                            out_chunk(prev[1], prev[2], prev[0].nt, mc2 - 1)
                if nt < NT - 1:
                    sums_finish(st)
                    y_sb = attn_end(st)
                    if prev is not None:
                        out_chunk(prev[1], prev[2], prev[0].nt, NTILE // P - 1)
                    prev = (st, y_sb, None)
                else:
                    # final tile: finish sums + mini-mms immediately after the
                    # last consume so the tail only waits on short DVE ops
                    sums_finish(st)
                    recip_last = rsum_mms(st)
                    y_last = attn_end(st)
                    if prev is not None:
                        out_chunk(prev[1], prev[2], prev[0].nt, NTILE // P - 1)

            # final tile out-projection: psum from the (now idle) pair pool so
            # all four chunks can be in flight at once; scales split across
            # DVE and Act, one wide store per pair
            ps_f0 = ps_pair_pool.tile([P, 2 * NTILE], F32, tag="ps_pair", name="ps_fin0")
            ps_f1 = ps_pair_pool.tile([P, 2 * NTILE], F32, tag="ps_pair", name="ps_fin1")
            halves = [ps_f0[:, :C], ps_f0[:, C:], ps_f1[:, :C], ps_f1[:, C:]]
            for j, ps_o in enumerate(halves):
                for ec in range(EC):
                    nc.tensor.matmul(
                        ps_o,
                        lhsT=y_last[:, ec, j * P : (j + 1) * P],
                        rhs=wo_sb[:, ec, :],
                        start=(ec == 0),
                        stop=(ec == EC - 1),
                    )
            for pair in range(2):
                o_sb = o_pool.tile([P, 2, C], F16, tag="o_sb")
                nc.vector.tensor_scalar_mul(
                    o_sb[:, 0, :], halves[2 * pair], recip_last[:, 2 * pair : 2 * pair + 1]
                )
                nc.scalar.activation(
                    o_sb[:, 1, :], halves[2 * pair + 1],
                    mybir.ActivationFunctionType.Copy,
                    scale=recip_last[:, 2 * pair + 1 : 2 * pair + 2],
                )
                row0 = (NT - 1) * NTILE + 2 * pair * P
                nc.sync.dma_start(
                    out_d[row0 : row0 + 2 * P, :].rearrange("(k p) c -> p k c", p=P),
                    o_sb,
                )

    nc.finalize()
    return nc


def _get_nc():
    if "nc" not in _CACHE:
        _CACHE["nc"] = _build()
    return _CACHE["nc"]


def kernel(x, wf, wg, wh, wo):
    x = np.asarray(x, dtype=np.float32)
    wf = np.asarray(wf, dtype=np.float32)
    wg = np.asarray(wg, dtype=np.float32)
    wh = np.asarray(wh, dtype=np.float32)
    wo = np.asarray(wo, dtype=np.float32)
    B = x.shape[0]
    assert x.shape == (B, 64, 64, C)

    wf2 = np.ascontiguousarray(
        np.concatenate([wf, wf], axis=1).astype(np.float16)
    )
    wg2 = np.ascontiguousarray(
        (0.25 * np.concatenate([wg, wg], axis=1)).astype(np.float16)
    )
    whs = np.ascontiguousarray((0.25 * wh).astype(np.float16))
    wo_c = np.ascontiguousarray(wo.astype(np.float16))

    nc = _get_nc()
    in_maps = []
    for b in range(B):
        xt = x[b].reshape(N, C).T.astype(np.float16)          # [512, 4096]
        xtq = np.ascontiguousarray(
            xt.reshape(C, 4, N // 4).transpose(1, 0, 2)        # [4, 512, 1024]
        )
        in_maps.append(
            {"xt": xtq, "wf2": wf2, "wg2": wg2, "whs": whs, "wo": wo_c}
        )

    res = run_bass_kernel_spmd(nc, in_maps, core_ids=list(range(B)))
    kernel.last_result = res

    out = np.empty((B, 64, 64, C), dtype=np.float32)
    for b in range(B):
        out[b] = res.results[b]["out"].astype(np.float32).reshape(64, 64, C)
    return out


# revision 22
# speedup vs baseline: 1.1954x; 1.1954x over previous
"""Pooled-KV attention block on 8 Trainium2 cores, data-parallel over batch.

Reference computation (per batch element b, with x_b: [64, 64, 512] -> [4096, 512]):
    f  = x_b @ wf                     # [4096, 64]
    xp = avgpool2x2(x_b)              # [1024, 512]
    g  = xp @ wg                      # [1024, 64]
    h  = xp @ wh                      # [1024, 256]
    a  = softmax(f @ g.T, axis=-1)    # [4096, 1024]
    y  = a @ h                        # [4096, 256]
    out = y @ wo                      # [4096, 512]

Kernel strategy (one core per batch element, weights replicated):
  - Host supplies x transposed and cast to fp16 (xT: [512, 4096]) so channel
    contractions have C on SBUF partitions; no on-device transposes anywhere.
  - All intermediates flow "transposed": fT [64, 4096], gT [64, 1024],
    h [m, 256] with m on partitions, scoresT [m, n], yT [e, n].
  - Matmul operands are fp16 (full PE rate, 10-bit mantissa, fp32 PSUM
    accumulation); final output is fp16 (upcast to fp32 on host).
  - Softmax skips max-subtraction (|scores| < ~6 for this data, exp is safe).
  - Row sums: exp tiles are pair-accumulated on DVE into acc [m,512], then
    four tiny N=1 matmuls (lhsT=acc slice, rhs=ones column) produce the sums
    already transposed to [n_chunk, 1] per-partition form -- no wide ones
    matmul and no DRAM transpose bounce.  Normalization stays folded into
    the out-projection copyback as a per-partition scalar multiply.
"""

import sys
import types

import numpy as np

import concourse.mybir as mybir
import concourse.tile as tile
from concourse import bacc
from concourse.bass_utils import run_bass_kernel_spmd

# If BASS_TRACE is set but this image's antenv lacks axon_hooks, bass_utils
# would crash on import; provide a no-op hook module so tracing degrades
# gracefully instead (a real hook installed earlier, e.g. by test.py, wins).
try:
    import antenv.axon_hooks  # noqa: F401
except ImportError:
    import antenv

    _stub = types.ModuleType("antenv.axon_hooks")
    _stub._hook = None
    _stub.set_axon_ntff_profile_hook = lambda h: setattr(_stub, "_hook", h)
    _stub.get_axon_ntff_profile_hook = lambda: _stub._hook
    sys.modules["antenv.axon_hooks"] = _stub
    antenv.axon_hooks = _stub

F32 = mybir.dt.float32
F16 = mybir.dt.float16

P = 128          # SBUF partitions
C = 512          # channels
KC = C // P      # 4 contraction chunks over channels
N = 4096         # query positions (64*64)
NTILE = 512      # n tile (psum free dim)
NT = N // NTILE  # 8 n tiles
M = 1024         # pooled key positions (32*32)
MC = M // P      # 8 key chunks
D = 64           # qk head dim
E = 256          # value dim (C//2)
EC = E // P      # 2 value chunks

_CACHE = {}


def _build():
    nc = bacc.Bacc(None, target_bir_lowering=False)

    xt_d = nc.dram_tensor("xt", [4, C, N // 4], F16, kind="ExternalInput")
    wf_d = nc.dram_tensor("wf2", [C, P], F16, kind="ExternalInput")   # [wf | wf]
    wg_d = nc.dram_tensor("wg2", [C, P], F16, kind="ExternalInput")   # 0.25*[wg | wg]
    wh_d = nc.dram_tensor("whs", [C, E], F16, kind="ExternalInput")   # 0.25*wh
    wo_d = nc.dram_tensor("wo", [E, C], F16, kind="ExternalInput")
    out_d = nc.dram_tensor("out", [N, C], F16, kind="ExternalOutput")

    with tile.TileContext(nc) as tc:
        with (
            tc.tile_pool(name="const", bufs=1) as const_pool,
            tc.tile_pool(name="ptmp", bufs=4) as ptmp_pool,
            tc.tile_pool(name="exp", bufs=4) as exp_pool,
            tc.tile_pool(name="sacc", bufs=2) as sacc_pool,
            tc.tile_pool(name="ysb", bufs=2) as y_pool,
            tc.tile_pool(name="osb", bufs=6) as o_pool,
            tc.tile_pool(name="small", bufs=4) as small_pool,
            tc.tile_pool(name="ps_pair", bufs=2, space="PSUM") as ps_pair_pool,
            tc.tile_pool(name="ps_work", bufs=1, space="PSUM") as ps_work_pool,
            tc.tile_pool(name="ps_y", bufs=1, space="PSUM") as ps_y_pool,
            tc.tile_pool(name="ps_rsum", bufs=1, space="PSUM") as ps_rsum_pool,
            tc.tile_pool(name="warm", bufs=1) as warm_pool,
        ):
            # ---- staged input load + f/g/h, by n-quarters of x ----
            # Each quarter of x (all channels, 1024 query positions) enables:
            # its pooling slice, two fT tiles, one gT quarter, two h chunks.
            # PE work starts when the first quarter lands instead of after the
            # full x load; DMA issue round-robins four HWDGE rings.
            xt_q = []
            for q in range(4):
                t = const_pool.tile([P, KC, N // 4], F16, name=f"xt_q{q}")
                xt_q.append(t)
            xp_q = []
            for q in range(4):
                t = const_pool.tile([P, KC, M // 4], F16, name=f"xp_q{q}")
                xp_q.append(t)
            wf_sb = const_pool.tile([P, KC, P], F16)
            wg_sb = const_pool.tile([P, KC, P], F16)
            wh_sb = const_pool.tile([P, KC, E], F16)
            wo_sb = const_pool.tile([P, EC, C], F16)
            ones_sb = const_pool.tile([P, 1], F16)
            fT_sb = const_pool.tile([P, N], F16)
            gT_sb = const_pool.tile([P, M], F16)
            h_sb = const_pool.tile([P, MC, E], F16)

            nc.gpsimd.memset(ones_sb, 1.0)
            warm_sb = warm_pool.tile([P, NTILE], F16)
            nc.gpsimd.memset(warm_sb, 1.0)

            NP = MC // 2  # score pairs per n tile

            def out_chunk(y_prev, recip_prev, nt_prev, j, on_act=False):
                ps_o = ps_work_pool.tile([P, C], F32, tag="ps_work", name=f"ps_o_{nt_prev}_{j}")
                for ec in range(EC):
                    nc.tensor.matmul(
                        ps_o,
                        lhsT=y_prev[:, ec, j * P : (j + 1) * P],
                        rhs=wo_sb[:, ec, :],
                        start=(ec == 0),
                        stop=(ec == EC - 1),
                    )
                o_sb = o_pool.tile([P, C], F16, tag="o_sb")
                if on_act:
                    nc.scalar.activation(
                        o_sb, ps_o, mybir.ActivationFunctionType.Copy,
                        scale=recip_prev[:, j : j + 1],
                    )
                else:
                    nc.vector.tensor_scalar_mul(o_sb, ps_o, recip_prev[:, j : j + 1])
                row0 = nt_prev * NTILE + j * P
                nc.sync.dma_start(out_d[row0 : row0 + P, :], o_sb)

            class TileState:
                pass

            def attn_begin(nt):
                st = TileState()
                st.nt = nt
                st.ps_y0 = ps_y_pool.tile([P, NTILE], F32, tag="ps_y0", name=f"ps_y0_{nt}")
                st.ps_y1 = ps_y_pool.tile([P, NTILE], F32, tag="ps_y1", name=f"ps_y1_{nt}")
                st.ets = {}
                st.et2s = {}
                return st

            def attn_scores(st, mc2):
                # two K=64 score matmuls packed into disjoint row groups,
                # writing the two banks of one psum tile; one wide exp
                nt = st.nt
                nsl = slice(nt * NTILE, (nt + 1) * NTILE)
                mcA, mcB = 2 * mc2, 2 * mc2 + 1
                ps_s2 = ps_pair_pool.tile([P, 2 * NTILE], F32, tag="ps_pair", name=f"ps_s2_{nt}_{mc2}")
                # 64x128 row tiling: the two K=64 matmuls run on independent
                # PE row-tiles T0 (SBUF 0-63) / T8 (SBUF 64-127) concurrently
                nc.tensor.matmul(
                    ps_s2[:, :NTILE],
                    lhsT=gT_sb[0:D, mcA * P : (mcA + 1) * P],
                    rhs=fT_sb[0:D, nsl],
                    start=True, stop=True,
                    tile_position=(0, 0),
                )
                nc.tensor.matmul(
                    ps_s2[:, NTILE:],
                    lhsT=gT_sb[D : 2 * D, mcB * P : (mcB + 1) * P],
                    rhs=fT_sb[D : 2 * D, nsl],
                    start=True, stop=True,
                    tile_position=(D, 0),
                )
                et2 = exp_pool.tile([P, 2 * NTILE], F16, tag="et", name=f"et2_{nt}_{mc2}")
                nc.scalar.activation(et2, ps_s2, mybir.ActivationFunctionType.Exp)
                st.ets[mc2] = (et2[:, :NTILE], et2[:, NTILE:])
                st.et2s[mc2] = et2

            def attn_consume(st, pc):
                first = pc == 0
                last = pc == NP - 1
                for k, et in enumerate(st.ets.pop(pc)):
                    mc = 2 * pc + k
                    nc.tensor.matmul(
                        st.ps_y0, lhsT=h_sb[:, mc, 0:P], rhs=et,
                        start=first and k == 0, stop=last and k == 1,
                    )
                    nc.tensor.matmul(
                        st.ps_y1, lhsT=h_sb[:, mc, P:E], rhs=et,
                        start=first and k == 0, stop=last and k == 1,
                    )

            def sums_finish(st):
                # t23 = et2 + et3; s2 = t01 + t23; acc = s2_lo + s2_hi
                nt = st.nt
                t23 = sacc_pool.tile([P, 2 * NTILE], F16, tag="t23", name=f"t23_{nt}")
                nc.vector.tensor_add(t23, st.et2s.pop(2), st.et2s.pop(3))
                s2 = sacc_pool.tile([P, 2 * NTILE], F16, tag="s2", name=f"s2_{nt}")
                nc.vector.tensor_add(s2, st.t01, t23)
                acc = sacc_pool.tile([P, NTILE], F16, tag="acc", name=f"acc_{nt}")
                nc.vector.tensor_add(acc, s2[:, :NTILE], s2[:, NTILE:])
                st.acc = acc

            def rsum_mms(st):
                # transposed row sums: rsum[n_j, 0] = sum_m acc[m, n_j]
                nt = st.nt
                ps_rs = ps_rsum_pool.tile([P, NTILE // P], F32, tag="ps_rsum", name=f"ps_rs_{nt}")
                for j in range(NTILE // P):
                    nc.tensor.matmul(
                        ps_rs[:, j : j + 1],
                        lhsT=st.acc[:, j * P : (j + 1) * P],
                        rhs=ones_sb,
                        start=True, stop=True,
                    )
                recip = small_pool.tile([P, NTILE // P], F32, tag="recip")
                nc.vector.reciprocal(recip, ps_rs)
                return recip

            def attn_end(st):
                y_sb = y_pool.tile([P, EC, NTILE], F16, tag="y_sb")
                nc.vector.tensor_copy(y_sb[:, 0, :], st.ps_y0)
                nc.scalar.copy(y_sb[:, 1, :], st.ps_y1)
                return y_sb

            NQ = N // 4   # 1024 query positions per quarter
            MQ = M // 4   # 128 pooled positions per quarter

            def load_quarter(q):
                # each quarter split across both HWDGE rings, one wide
                # descriptor per ring (dma_start issue cost is ~0.6 us each);
                # host supplies x quarter-contiguous so reads are sequential
                nc.sync.dma_start(
                    xt_q[q][:, 0:2, :],
                    xt_d[q, 0 : 2 * P, :].rearrange("(kc p) n -> p kc n", p=P),
                )
                nc.scalar.dma_start(
                    xt_q[q][:, 2:4, :],
                    xt_d[q, 2 * P : 4 * P, :].rearrange("(kc p) n -> p kc n", p=P),
                )

            # small weights first (first f/g matmuls need them); wh/wo are not
            # needed until pooling / the first out-projection, so they ride
            # the slow gpsimd software-DGE ring in parallel with x
            nc.sync.dma_start(wf_sb, wf_d.rearrange("(kc p) d -> p kc d", p=P))
            nc.scalar.dma_start(wg_sb, wg_d.rearrange("(kc p) d -> p kc d", p=P))
            nc.gpsimd.dma_start(wo_sb, wo_d.rearrange("(ec p) c -> p ec c", p=P))
            load_quarter(0)
            nc.scalar.dma_start(wh_sb, wh_d.rearrange("(kc p) e -> p kc e", p=P))
            for q in range(1, 4):
                load_quarter(q)

            # PE warmup: the first ~9 us are DMA-dead (ring init + x in
            # flight); dummy matmuls keep the HAM activity monitor from
            # holding the PE at half clock when real work arrives
            ps_warm = ps_work_pool.tile([P, C], F32, tag="ps_work", name="ps_warm")
            for w in range(34):
                nc.tensor.matmul(
                    ps_warm, lhsT=warm_sb[:, 0:P], rhs=warm_sb,
                    start=True, stop=True,
                )

            for q in range(4):
                # pooling for quarter q: n = 256*i + 64*a + 2*j + b over all kc
                # t0 on DVE, t1 on GpSimd so the two halves run concurrently
                xv = xt_q[q].rearrange(
                    "p kc (i a j b) -> p kc i a j b", i=8, a=2, j=32, b=2
                )
                t0 = ptmp_pool.tile([P, KC, 8, 32], F16, tag="pool_t0")
                nc.vector.tensor_add(t0, xv[:, :, :, 0, :, 0], xv[:, :, :, 0, :, 1])
                t1 = ptmp_pool.tile([P, KC, 8, 32], F16, tag="pool_t1")
                nc.vector.tensor_add(t1, xv[:, :, :, 1, :, 0], xv[:, :, :, 1, :, 1])
                nc.vector.tensor_add(
                    xp_q[q].rearrange("p kc (i j) -> p kc i j", i=8), t0, t1
                )

                # fT tiles for this quarter (two n tiles of 512)
                for half in range(2):
                    nt = 2 * q + half
                    ps_w = ps_pair_pool.tile([P, 2 * NTILE], F32, tag="ps_pair")
                    ps = ps_w[:, :NTILE]
                    for kc in range(KC):
                        nc.tensor.matmul(
                            ps,
                            lhsT=wf_sb[:, kc, :],
                            rhs=xt_q[q][:, kc, half * NTILE : (half + 1) * NTILE],
                            start=(kc == 0),
                            stop=(kc == KC - 1),
                        )
                    nc.scalar.copy(fT_sb[:, nt * NTILE : (nt + 1) * NTILE], ps)

                # gT quarter (128 key columns)
                ps_w = ps_pair_pool.tile([P, 2 * NTILE], F32, tag="ps_pair")
                ps = ps_w[:, :MQ]
                for kc in range(KC):
                    nc.tensor.matmul(
                        ps,
                        lhsT=wg_sb[:, kc, :],
                        rhs=xp_q[q][:, kc, :],
                        start=(kc == 0),
                        stop=(kc == KC - 1),
                    )
                nc.scalar.copy(gT_sb[:, q * MQ : (q + 1) * MQ], ps)

                # h chunks for this quarter (mc = 2q, 2q+1)
                for half in range(2):
                    mc = 2 * q + half
                    ps_w = ps_pair_pool.tile([P, 2 * NTILE], F32, tag="ps_pair")
                    ps = ps_w[:, :E]
                    for kc in range(KC):
                        nc.tensor.matmul(
                            ps,
                            lhsT=xp_q[q][:, kc, half * P : (half + 1) * P],
                            rhs=wh_sb[:, kc, :],
                            start=(kc == 0),
                            stop=(kc == KC - 1),
                        )
                    nc.scalar.copy(h_sb[:, mc, :], ps)

            # ---- attention, software-pipelined ----
            # Tile nt's scores/exp/y run while tile nt-1 finishes: its
            # transposed row-sum mini-matmuls + reciprocal land at the top of
            # tile nt, and its four out-projection chunks interleave between
            # tile nt's score/consume steps.

            prev = None
            for nt in range(NT):
                st = attn_begin(nt)
                for mc2 in range(NP + 1):
                    if mc2 < NP:
                        attn_scores(st, mc2)
                    if mc2 == 0 and prev is not None:
                        prev_recip = rsum_mms(prev[0])
                        prev = (prev[0], prev[1], prev_recip)
                    if mc2 >= 1:
                        attn_consume(st, mc2 - 1)
                        if mc2 == 2:
                            t01 = sacc_pool.tile([P, 2 * NTILE], F16, tag="t01", name=f"t01_{nt}")
                            nc.vector.tensor_add(t01, st.et2s.pop(0), st.et2s.pop(1))
                            st.t01 = t01
                        if prev is not None and 1 <= mc2 <= NP - 1:
                            out_chunk(prev[1], prev[2], prev[0].nt, mc2 - 1,
                                      on_act=(mc2 == NP - 1))
                if nt < NT - 1:
                    sums_finish(st)
                    y_sb = attn_end(st)
                    if prev is not None:
                        out_chunk(prev[1], prev[2], prev[0].nt, NTILE // P - 1)
                    prev = (st, y_sb, None)
                else:
                    # final tile: finish sums + mini-mms immediately after the
                    # last consume so the tail only waits on short DVE ops
                    sums_finish(st)
                    recip_last = rsum_mms(st)
                    y_last = attn_end(st)
                    if prev is not None:
                        out_chunk(prev[1], prev[2], prev[0].nt, NTILE // P - 1)

            # final tile out-projection: psum from the (now idle) pair pool so
            # all four chunks can be in flight at once; scales split across
            # DVE and Act, one wide store per pair
            ps_f0 = ps_pair_pool.tile([P, 2 * NTILE], F32, tag="ps_pair", name="ps_fin0")
            ps_f1 = ps_pair_pool.tile([P, 2 * NTILE], F32, tag="ps_pair", name="ps_fin1")
            halves = [ps_f0[:, :C], ps_f0[:, C:], ps_f1[:, :C], ps_f1[:, C:]]
            for j, ps_o in enumerate(halves):
                for ec in range(EC):
                    nc.tensor.matmul(
                        ps_o,
                        lhsT=y_last[:, ec, j * P : (j + 1) * P],
                        rhs=wo_sb[:, ec, :],
                        start=(ec == 0),
                        stop=(ec == EC - 1),
                    )
            for j, ps_o in enumerate(halves):
                o_sb = o_pool.tile([P, C], F16, tag="o_sb")
                if j % 2 == 0:
                    nc.vector.tensor_scalar_mul(o_sb, ps_o, recip_last[:, j : j + 1])
                else:
                    nc.scalar.activation(
                        o_sb, ps_o, mybir.ActivationFunctionType.Copy,
                        scale=recip_last[:, j : j + 1],
                    )
                row0 = (NT - 1) * NTILE + j * P
                nc.sync.dma_start(out_d[row0 : row0 + P, :], o_sb)

    nc.finalize()
    return nc


def _get_nc():
    if "nc" not in _CACHE:
        _CACHE["nc"] = _build()
    return _CACHE["nc"]


def kernel(x, wf, wg, wh, wo):
    x = np.asarray(x, dtype=np.float32)
    wf = np.asarray(wf, dtype=np.float32)
    wg = np.asarray(wg, dtype=np.float32)
    wh = np.asarray(wh, dtype=np.float32)
    wo = np.asarray(wo, dtype=np.float32)
    B = x.shape[0]
    assert x.shape == (B, 64, 64, C)

    wf2 = np.ascontiguousarray(
        np.concatenate([wf, wf], axis=1).astype(np.float16)
    )
    wg2 = np.ascontiguousarray(
        (0.25 * np.concatenate([wg, wg], axis=1)).astype(np.float16)
    )
    whs = np.ascontiguousarray((0.25 * wh).astype(np.float16))
    wo_c = np.ascontiguousarray(wo.astype(np.float16))

    nc = _get_nc()
    in_maps = []
    for b in range(B):
        xt = x[b].reshape(N, C).T.astype(np.float16)          # [512, 4096]
        xtq = np.ascontiguousarray(
            xt.reshape(C, 4, N // 4).transpose(1, 0, 2)        # [4, 512, 1024]
        )
        in_maps.append(
            {"xt": xtq, "wf2": wf2, "wg2": wg2, "whs": whs, "wo": wo_c}
        )

    res = run_bass_kernel_spmd(nc, in_maps, core_ids=list(range(B)))
    kernel.last_result = res

    out = np.empty((B, 64, 64, C), dtype=np.float32)
    for b in range(B):
        out[b] = res.results[b]["out"].astype(np.float32).reshape(64, 64, C)
    return out
